# revision 22
# baseline (speedup 1.0000x reference)
"""Trainium2 Bass kernel for a multi-head cross-attention module.

Math (validated vs reference to 5e-7 in f32):
  Q = x@Wq+bq, K = x@Wk+bk  (N=2048, 8 heads, head_dim=64)
  scores[q,k,h] = <Q[q,h,:], K[k,h,:]>/8       (spatial bias is a softmax
                                                shift along k -> provably a
                                                no-op, skipped)
  A = softmax_k(scores); out[q] = sum_{k,h} A[q,k,h]*U[k,h] + bo
  where U[k,h] = mg[k] * (x[k]@Wv_tilde[:,h] + bv_tilde[h]) folds the V
  projection, motion gate and output projection into one (N,8) matrix:
    Wv_tilde[c,h] = sum_d Wv[c,h*64+d]*Wo[h*64+d],  bv_tilde likewise.

Sharding: queries split 256/core across 8 cores; K/U replicated.
Per core: scores computed transposed ST_h[k,q] (k on partitions) so both
Z = sum_k exp and W = sum_k exp*U are PE column-sum matmuls against the
stationary [ones | U] matrix.  exp without max-subtraction (max|S| < 3).

Structural constraint honored throughout: this toolchain's walrus accepts
only ONE sync wait per lowered compute instruction (LDWEIGHTS and MATMUL
each get one slot).  Hence: weights feeding PE go through DVE staging or
arrive on the lhsT (LW) side only; per-key-tile buffers are dedicated (no
slot reuse WARs); ACT applies the motion gate via copy-with-scale so its
dep on the sigmoid is same-engine; the Z/W PSUM accumulator is first
touched by zeroing matmuls whose single wait absorbs the freed-bank zone
deps; DMA'd per-partition bias vectors get an early DVE "touch" so their
consumers' DMA tick is already in the DVE clock.
"""

import numpy as np
import ml_dtypes
from contextlib import ExitStack

import concourse.bass as bass
import concourse.mybir as mybir
import concourse.tile as tile
from concourse import masks
from concourse.bass_utils import run_bass_kernel_spmd

N = 2048
CIN = 256
DOUT = 512
H = 8
HD = 64
NCORES = 8
NQ = N // NCORES        # 256 queries per core
NKT = N // 128          # 16 key tiles
F32 = mybir.dt.float32
BF16 = mybir.dt.bfloat16

_CACHE = {}


def _build_nc(legalize=True):
    nc = bass.Bass()
    d_x = nc.declare_dram_parameter("x", [N, CIN], F32, isOutput=False)
    d_xq = nc.declare_dram_parameter("xq", [NQ, CIN], F32, isOutput=False)
    d_wq = nc.declare_dram_parameter("wq_bf", [CIN, DOUT], BF16, isOutput=False)
    d_wk = nc.declare_dram_parameter("wk_bf", [CIN, DOUT], BF16, isOutput=False)
    d_wv = nc.declare_dram_parameter("wv_bf", [128, 18], BF16, isOutput=False)
    d_bva = nc.declare_dram_parameter("bv_aug", [1, 9], BF16, isOutput=False)
    d_wm1 = nc.declare_dram_parameter("wmg1_bf", [2, HD], BF16, isOutput=False)
    d_wm2 = nc.declare_dram_parameter("wmg2_bf", [HD, 1], BF16, isOutput=False)
    d_bq = nc.declare_dram_parameter("bq_col", [128, 4], F32, isOutput=False)
    d_bk = nc.declare_dram_parameter("bk_col", [128, 4], F32, isOutput=False)
    d_bm1 = nc.declare_dram_parameter("bmg1_col", [HD, 1], F32, isOutput=False)
    d_bm2 = nc.declare_dram_parameter("bmg2_rep", [128, 1], F32, isOutput=False)
    d_bo = nc.declare_dram_parameter("bo_rep", [128, 1], F32, isOutput=False)
    d_mf = nc.declare_dram_parameter("mf", [2, N], F32, isOutput=False)
    d_out = nc.declare_dram_parameter("out", [NQ, 1], F32, isOutput=True)

    with tile.TileContext(nc) as tc:
        with ExitStack() as ctx:
            _body(ctx, tc, d_x, d_xq, d_wq, d_wk, d_wv, d_bva, d_wm1, d_wm2,
                  d_bq, d_bk, d_bm1, d_bm2, d_bo, d_mf, d_out)
    if legalize:
        _legalize_waits(nc)
    return nc


def _legalize_waits(nc):
    """walrus accepts a single sync wait per lowered instruction; split any
    extra waits onto injected same-engine NoOps placed just before."""
    cnt = 0
    skip = ("InstEventSemaphore", "InstNoOp", "InstISA")
    for f in nc.m.functions:
        for bb in f.blocks:
            out = []
            for ins in bb.instructions:
                si = getattr(ins, "sync_info", None)
                waits = list(si.on_wait) if (si is not None and si.on_wait) else []
                if len(waits) >= 2 and type(ins).__name__ not in skip:
                    for w in waits[:-1]:
                        nop = mybir.InstEventSemaphore(
                            name=f"wsplit_{cnt}", ins=[], outs=[])
                        cnt += 1
                        nop.engine = ins.engine
                        nop.sync_info = mybir.SyncInfo(on_wait=[w], on_update=[])
                        out.append(nop)
                    ins.sync_info = mybir.SyncInfo(
                        on_wait=[waits[-1]], on_update=list(si.on_update or []))
                out.append(ins)
            bb.instructions[:] = out
    return nc


def _body(ctx, tc, d_x, d_xq, d_wq, d_wk, d_wv, d_bva, d_wm1, d_wm2,
          d_bq, d_bk, d_bm1, d_bm2, d_bo, d_mf, d_out):
    nc = tc.nc
    AF = mybir.ActivationFunctionType
    OP = mybir.AluOpType

    const_pool = ctx.enter_context(tc.tile_pool(name="const", bufs=1))
    persist = ctx.enter_context(tc.tile_pool(name="persist", bufs=1))
    ld_pool = ctx.enter_context(tc.tile_pool(name="ld", bufs=4))
    xload = ctx.enter_context(tc.tile_pool(name="xload", bufs=1))

    ident = const_pool.tile([128, 128], F32)
    masks.make_identity(nc, ident[:])
    ident_bf = const_pool.tile([128, 128], BF16)
    masks.make_identity(nc, ident_bf[:])

    # ---- constant loads ----
    bq_col = const_pool.tile([128, 4], F32)
    nc.sync.dma_start(bq_col[:], d_bq[:])
    bk_col = const_pool.tile([128, 4], F32)
    nc.sync.dma_start(bk_col[:], d_bk[:])
    bm1_col = const_pool.tile([HD, 1], F32)
    nc.sync.dma_start(bm1_col[:], d_bm1[:])
    bm2_rep = const_pool.tile([128, 1], F32)
    nc.sync.dma_start(bm2_rep[:], d_bm2[:])
    bo_rep = const_pool.tile([128, 1], F32)
    nc.sync.dma_start(bo_rep[:], d_bo[:])
    wv_ld = const_pool.tile([128, 18], BF16)
    nc.sync.dma_start(wv_ld[:], d_wv[:])
    bva_ld = const_pool.tile([1, 9], BF16)
    nc.sync.dma_start(bva_ld[:], d_bva[:])
    wm1_ld = const_pool.tile([2, HD], BF16)
    nc.sync.dma_start(wm1_ld[:], d_wm1[:])
    wm2_ld = const_pool.tile([HD, 1], BF16)
    nc.sync.dma_start(wm2_ld[:], d_wm2[:])
    mf_sb = const_pool.tile([2, N], F32)
    nc.sync.dma_start(mf_sb[:], d_mf[:])
    wq_bf = [const_pool.tile([128, DOUT], BF16, name=f"wq{c}", tag=f"wq{c}")
             for c in range(2)]
    wk_bf = [const_pool.tile([128, DOUT], BF16, name=f"wk{c}", tag=f"wk{c}")
             for c in range(2)]
    for c in range(2):
        nc.sync.dma_start(wq_bf[c][:], d_wq[c * 128:(c + 1) * 128, :])
        nc.sync.dma_start(wk_bf[c][:], d_wk[c * 128:(c + 1) * 128, :])

    # ---- persistent activations / staged weights ----
    xT = [persist.tile([128, N], BF16, name=f"xT{c}", tag=f"xT{c}")
          for c in range(2)]
    xqT = [persist.tile([128, NQ], BF16, name=f"xqT{c}", tag=f"xqT{c}")
           for c in range(2)]
    KT = [persist.tile([128, N], BF16, name=f"KT{d}", tag=f"KT{d}")
          for d in range(4)]
    QT = [persist.tile([128, NQ], BF16, name=f"QT{d}", tag=f"QT{d}")
          for d in range(4)]
    uw = persist.tile([128, 9 * NKT], BF16)   # [1 | U_0..U_7] per key tile
    mg_col = persist.tile([128, NKT], F32)
    mf_bf = persist.tile([2, N], BF16)
    h1_bf = persist.tile([HD, N], BF16)
    mgp_sb = persist.tile([1, N], F32)
    zw_sb = persist.tile([9, N], F32)
    wv_bf = persist.tile([128, 18], BF16)
    bva_bf = persist.tile([1, 9], BF16)
    wm1_bf = persist.tile([2, HD], BF16)
    wm2_bf = persist.tile([HD, 1], BF16)
    ones_row = persist.tile([1, 128], BF16)
    zeros9 = persist.tile([1, 9], BF16)
    scraps = [persist.tile([128, 1], F32, name=f"scrap{i}", tag=f"scrap{i}")
              for i in range(9)]

    # DVE staging copies + touches: pull every DMA completion into the DVE
    # clock early, and hand PE-facing weights a DVE producer.
    nc.vector.tensor_copy(mf_bf[:], mf_sb[:])
    nc.vector.tensor_copy(wv_bf[:], wv_ld[:])
    nc.vector.tensor_copy(bva_bf[:], bva_ld[:])
    nc.vector.tensor_copy(wm1_bf[:], wm1_ld[:])
    nc.vector.tensor_copy(wm2_bf[:], wm2_ld[:])
    nc.vector.memset(ones_row[:], 1.0)
    nc.vector.memset(zeros9[:], 0.0)
    nc.vector.tensor_copy(scraps[0][:], bo_rep[:])
    nc.vector.tensor_copy(scraps[1][:], bq_col[:, 0:1])
    nc.vector.tensor_copy(scraps[2][:], bk_col[:, 0:1])
    nc.vector.tensor_copy(scraps[3][0:HD, :], bm1_col[:])
    nc.vector.tensor_copy(scraps[4][:], bm2_rep[:])
    # ACT warm-up: absorbs the const-AP (immediate bias) dependency.
    actw = const_pool.tile([2, 1], F32)
    nc.scalar.activation(actw[:], mf_bf[0:2, 0:1], AF.Exp, bias=0.0, scale=1.0)

    pu_tiles = []

    # ======== phase 1: transposes + projections ========
    with tc.tile_pool(name="ps1", bufs=4, space="PSUM") as ps1:
        # dummy transposes: consume the gpsimd(identity) dep once per dtype
        warm = ps1.tile([128, 512], BF16, tag="ps1b", bufs=2)
        nc.tensor.transpose(warm[:, 0:128], ident_bf[:], ident_bf[:])
        warm2 = ps1.tile([128, 512], F32, tag="ps1", bufs=3)
        nc.tensor.transpose(warm2[:, 0:128], ident[:], ident[:])

        # x -> xT (bf16) via DVE cast then PE transpose
        for kt in range(NKT):
            xraw = xload.tile([128, CIN], F32, name=f"xraw{kt}", tag=f"xraw{kt}")
            nc.sync.dma_start(xraw[:], d_x[kt * 128:(kt + 1) * 128, :])
            xrb = xload.tile([128, CIN], BF16, name=f"xrb{kt}", tag=f"xrb{kt}")
            nc.vector.tensor_copy(xrb[:], xraw[:])
            for c in range(2):
                tp = ps1.tile([128, 512], BF16, tag="ps1b", bufs=2)
                nc.tensor.transpose(tp[:, 0:128], xrb[:, c * 128:(c + 1) * 128],
                                    ident_bf[:])
                nc.vector.tensor_copy(xT[c][:, kt * 128:(kt + 1) * 128],
                                      tp[:, 0:128])
        for qt in range(2):
            xraw = xload.tile([128, CIN], F32, name=f"xqraw{qt}", tag=f"xqraw{qt}")
            nc.sync.dma_start(xraw[:], d_xq[qt * 128:(qt + 1) * 128, :])
            xrb = xload.tile([128, CIN], BF16, name=f"xqrb{qt}", tag=f"xqrb{qt}")
            nc.vector.tensor_copy(xrb[:], xraw[:])
            for c in range(2):
                tp = ps1.tile([128, 512], BF16, tag="ps1b", bufs=2)
                nc.tensor.transpose(tp[:, 0:128], xrb[:, c * 128:(c + 1) * 128],
                                    ident_bf[:])
                nc.vector.tensor_copy(xqT[c][:, qt * 128:(qt + 1) * 128],
                                      tp[:, 0:128])

        # K^T = Wk^T @ x^T  (4 dout tiles x 4 free chunks, 2 c-chunk accum)
        for d in range(4):
            for f in range(4):
                pp = ps1.tile([128, 512], F32, tag="ps1", bufs=3)
                for c in range(2):
                    nc.tensor.matmul(pp[:], wk_bf[c][:, d * 128:(d + 1) * 128],
                                     xT[c][:, f * 512:(f + 1) * 512],
                                     start=(c == 0), stop=(c == 1))
                nc.vector.tensor_scalar_add(KT[d][:, f * 512:(f + 1) * 512],
                                            pp[:], bk_col[:, d:d + 1])
            pq = ps1.tile([128, 512], F32, tag="ps1", bufs=3)
            for c in range(2):
                nc.tensor.matmul(pq[:, 0:NQ], wq_bf[c][:, d * 128:(d + 1) * 128],
                                 xqT[c][:], start=(c == 0), stop=(c == 1))
            nc.vector.tensor_scalar_add(QT[d][:], pq[:, 0:NQ], bq_col[:, d:d + 1])

        # motion gate: H1^T = relu(Wmg1^T @ mf + bmg1)  [64, N]
        for f in range(4):
            ph = ps1.tile([128, 512], F32, tag="ps1", bufs=3)
            nc.tensor.matmul(ph[0:HD, :], wm1_bf[:],
                             mf_bf[:, f * 512:(f + 1) * 512])
            nc.vector.tensor_scalar(h1_bf[:, f * 512:(f + 1) * 512], ph[0:HD, :],
                                    bm1_col[:], 0.0, op0=OP.add, op1=OP.max)
        # mg_pre = Wmg2^T @ H1 + bmg2  [1, N]
        for f in range(4):
            pm = ps1.tile([128, 512], F32, tag="ps1", bufs=3)
            nc.tensor.matmul(pm[0:1, :], wm2_bf[:],
                             h1_bf[:, f * 512:(f + 1) * 512])
            nc.vector.tensor_scalar_add(mgp_sb[:, f * 512:(f + 1) * 512],
                                        pm[0:1, :], bm2_rep[0:1, 0:1])
        # transpose to [128, 16] then sigmoid
        pmc = ps1.tile([128, 512], F32, tag="pmc", bufs=1)
        for kt in range(NKT):
            nc.tensor.transpose(pmc[:, kt:kt + 1],
                                mgp_sb[0:1, kt * 128:(kt + 1) * 128],
                                ident[0:1, 0:1])
        nc.scalar.activation(mg_col[:], pmc[:, 0:NKT], AF.Sigmoid,
                             bias=0.0, scale=1.0)

        # U-block: pu[k, 0:9] = [1 | x@Wv_t + bv_t] via [x|1]@[[0,Wv],[1,bv]]
        # Two alternating single-bank PSUM tiles; each kt writes its own
        # column block, so no cross-engine WAR chains appear.
        pu_ab = [ps1.tile([128, (NKT // 2) * 9], F32, tag=f"u0{i}", bufs=1,
                          name=f"pu{i}") for i in range(2)]
        for kt in range(NKT):
            pu = pu_ab[kt % 2]
            o = (kt // 2) * 9
            for c in range(2):
                nc.tensor.matmul(pu[:, o:o + 9],
                                 xT[c][:, kt * 128:(kt + 1) * 128],
                                 wv_bf[:, c * 9:(c + 1) * 9],
                                 start=(c == 0), stop=False)
            nc.tensor.matmul(pu[:, o:o + 9], ones_row[:], bva_bf[:],
                             start=False, stop=True)
            # uw columns via ACT: col0 copy, cols1-8 scaled by mg (same-engine
            # dep on the sigmoid -> single PE wait)
            nc.scalar.activation(uw[:, kt * 9:kt * 9 + 1], pu[:, o:o + 1],
                                 AF.Copy, bias=0.0, scale=1.0)
            nc.scalar.activation(uw[:, kt * 9 + 1:kt * 9 + 9], pu[:, o + 1:o + 9],
                                 AF.Copy, bias=0.0, scale=mg_col[:, kt:kt + 1])
        # leave the u0/pmc banks with a DVE last-touch
        nc.vector.tensor_copy(scraps[5][:], pu_ab[0][:, 0:1])
        nc.vector.tensor_copy(scraps[6][:], pu_ab[1][:, 0:1])
        nc.vector.tensor_copy(scraps[7][:], pmc[:, 0:1])

    # ======== phase 2: scores -> exp -> Z/W accumulation ========
    with tc.tile_pool(name="zwp", bufs=1, space="PSUM") as zwp, \
         tc.tile_pool(name="stp", bufs=3, space="PSUM") as stp, \
         tc.tile_pool(name="pp", bufs=1) as pp:
        zw_ps = zwp.tile([9, N], F32)
        # zero the accumulator; these first touches absorb the freed-bank
        # zone deps with a single wait each
        for g in range(4):
            nc.tensor.matmul(zw_ps[:, g * 512:(g + 1) * 512], zeros9[:],
                             xT[0][0:1, 0:512], start=True, stop=False)
        for kt in range(NKT):
            p_sb = pp.tile([128, H * NQ], BF16, name=f"p{kt}", tag=f"p{kt}")
            for d in range(4):              # head pair (2d, 2d+1)
                for hh in range(2):
                    h = 2 * d + hh
                    st = stp.tile([128, NQ], F32, tag="st")
                    nc.tensor.matmul(
                        st[:],
                        KT[d][hh * HD:(hh + 1) * HD, kt * 128:(kt + 1) * 128],
                        QT[d][hh * HD:(hh + 1) * HD, :],
                    )
                    nc.scalar.activation(p_sb[:, h * NQ:(h + 1) * NQ], st[:],
                                         AF.Exp, scale=0.125)
            for g in range(4):              # 512-wide Z/W accumulation chunks
                nc.tensor.matmul(zw_ps[:, g * 512:(g + 1) * 512],
                                 uw[:, kt * 9:kt * 9 + 9],
                                 p_sb[:, g * 512:(g + 1) * 512],
                                 start=False, stop=(kt == NKT - 1))

        # ======== phase 3: final combine ========
        nc.vector.tensor_copy(zw_sb[:], zw_ps[:])
        zt_ps = zwp.tile([128, 9 * NKT], F32, tag="zwT")
        for i in range(NKT):                # chunk i: head i//2, query half i%2
            nc.tensor.transpose(zt_ps[:, i * 9:i * 9 + 9],
                                zw_sb[:, i * 128:(i + 1) * 128], ident[0:9, 0:9])
        res = ld_pool.tile([128, 2], F32, tag="res")
        for qh in range(2):
            zr = ld_pool.tile([128, H], F32, tag="zr")
            nc.vector.reciprocal(zr[:], zt_ps[:, 9 * qh:9 * qh + 18 * 7 + 1:18])
            wz = ld_pool.tile([128, H], F32, tag="wz")
            nc.vector.tensor_mul(wz[:],
                                 zt_ps[:, 9 * qh + 1:9 * qh + 1 + 19 * 7 + 1:19],
                                 zr[:])
            sm = ld_pool.tile([128, 1], F32, tag="sm")
            nc.vector.reduce_sum(sm[:], wz[:], axis=mybir.AxisListType.X)
            nc.vector.tensor_scalar_add(res[:, qh:qh + 1], sm[:], bo_rep[:])
        nc.sync.dma_start(d_out.rearrange("(q p) o -> p (q o)", p=128), res[:])


def _host_prep(inputs):
    f32 = np.float32
    bf = ml_dtypes.bfloat16
    x = np.ascontiguousarray(inputs["x"], dtype=f32)
    Wo0 = inputs["Wo"][:, 0].astype(f32)
    wv_t = (inputs["Wv"].astype(f32) * Wo0[None, :]).reshape(CIN, H, HD).sum(-1)
    bv_t = (inputs["bv"].astype(f32) * Wo0).reshape(H, HD).sum(-1)
    # wv_bf: [128, 18] = two c-chunks side by side, each [0 | Wv_t chunk]
    wv_aug = np.zeros((CIN, 9), f32)
    wv_aug[:, 1:9] = wv_t
    wv_pack = wv_aug.reshape(2, 128, 9).transpose(1, 0, 2).reshape(128, 18)
    bv_aug = np.zeros((1, 9), f32)
    bv_aug[0, 0] = 1.0
    bv_aug[0, 1:9] = bv_t
    common = dict(
        x=x,
        wq_bf=inputs["Wq"].astype(bf),
        wk_bf=inputs["Wk"].astype(bf),
        wv_bf=np.ascontiguousarray(wv_pack).astype(bf),
        bv_aug=np.ascontiguousarray(bv_aug).astype(bf),
        wmg1_bf=inputs["Wmg1"].astype(bf),
        wmg2_bf=inputs["Wmg2"].astype(bf),
        bq_col=np.ascontiguousarray(inputs["bq"].astype(f32).reshape(4, 128).T),
        bk_col=np.ascontiguousarray(inputs["bk"].astype(f32).reshape(4, 128).T),
        bmg1_col=np.ascontiguousarray(inputs["bmg1"].astype(f32).reshape(HD, 1)),
        bmg2_rep=np.full((128, 1), inputs["bmg2"][0], f32),
        bo_rep=np.full((128, 1), inputs["bo"][0], f32),
        mf=np.ascontiguousarray(
            np.stack([inputs["rel_vel"][:, 0],
                      inputs["rel_angle"][:, 0]]).astype(f32)),
    )
    return common


def kernel(**inputs):
    if "nc" not in _CACHE:
        _CACHE["nc"] = _build_nc()
    nc = _CACHE["nc"]
    common = _host_prep(inputs)
    x = common["x"]
    in_maps = [dict(common, xq=np.ascontiguousarray(x[i * NQ:(i + 1) * NQ]))
               for i in range(NCORES)]
    res = run_bass_kernel_spmd(nc, in_maps, core_ids=list(range(NCORES)),
                               **_CACHE.get("run_kwargs", {}))
    _CACHE["last_results"] = res
    out = np.concatenate([np.asarray(res.results[i]["out"])[:, 0]
                          for i in range(NCORES)])
    return out.astype(np.float32)
